# revision 28
# baseline (speedup 1.0000x reference)
"""Trainium2 Bass kernel for nn_BoundaryLoss_49306224558104.

Math note: in the reference, every pixel is either foreground (where
neg = edt(~fg) is exactly 0) or background (where pos = edt(fg) is
exactly 0), so min(pos, neg) == 0 at every pixel and dist_map is
identically zero (bitwise-exact in f32: the EDT of a pixel whose own
d0 is 0 takes the y==j / k==i branch with cost 0, and sqrt(0) == 0).
The loss therefore reduces exactly to mean(softplus(x) - x*z) with
x = pred.squeeze(1), z = (target > 0).

Sharding: pure data-parallel — sample b goes to core b (B == 8 ==
n_cores). Per core, the sample's pred (f32) and target (cast to f32
on host) are packed into one [128, 1024] DRAM buffer, DMA'd on the
sync HWDGE ring (the scalar ring stays free so the ACT PWP-table
load, forced early by a dummy activation, doesn't queue behind a
transfer). softplus(x) = ln(1 + exp(x)) on the scalar engine (inputs
are N(0,1) logits, |x| << 88, so the direct form neither overflows
nor loses precision; this build's act tables have exp+ln in one set
but no softplus table at all). Row sums come from the activation /
scalar_tensor_tensor accumulators; a ones-vector matmul on the
tensor engine collapses the 128 partition partials to a single
[1, 2] PSUM value so the output DMA is one 8-byte descriptor (a
[128, 1] per-partition DMA costs ~7 us in descriptor processing).
The compiler-injected teardown drains retire the in-flight output
DMA (~2 us HBM write receipt). Host combines the 8 x [1, 2] partials
into the scalar mean. Measured: ~15 us HW exec (from ~23.7 us for
the naive version), relative error 0.0 vs the f32 reference value.
"""

import numpy as np

B, H, W = 8, 256, 256
P, F = 128, 512  # H*W == P*F
FX2 = 2 * F
N_CORES = 8


def _build_nc():
    import concourse.bass as bass
    import concourse.mybir as mybir

    nc = bass.Bass(trn_type="TRN2")

    xt = nc.declare_dram_parameter("xt", [P, FX2], mybir.dt.float32, isOutput=False)
    out = nc.declare_dram_parameter("out", [1, 2], mybir.dt.float32, isOutput=True)

    zeros128 = nc.const_aps.aps[(mybir.dt.float32, 0.0)]  # [128,1] framework const
    ones128 = nc.const_aps.aps[(mybir.dt.float32, 1.0)]  # [128,1] framework const

    with (
        nc.sbuf_tensor("xtt", [P, FX2], mybir.dt.float32) as xtt,
        nc.sbuf_tensor("e", [P, F], mybir.dt.float32) as e,
        nc.sbuf_tensor("l", [P, F], mybir.dt.float32) as l,
        nc.sbuf_tensor("xz", [P, F], mybir.dt.float32) as xz,
        nc.sbuf_tensor("sums", [P, 2], mybir.dt.float32) as sums,
        nc.sbuf_tensor("trash", [P, 1], mybir.dt.float32) as trash,
        nc.sbuf_tensor("res", [1, 2], mybir.dt.float32) as res,
        nc.psum_tensor("ps", [1, 2], mybir.dt.float32) as ps,
        nc.psum_tensor("ps_warm", [1, 2], mybir.dt.float32) as ps_warm,
        nc.semaphore("x_sem") as x_sem,
        nc.semaphore("x2_sem") as x2_sem,
        nc.semaphore("t_sem") as t_sem,
        nc.semaphore("s_sem") as s_sem,
        nc.semaphore("a_sem") as a_sem,
        nc.semaphore("v_sem") as v_sem,
        nc.semaphore("m_sem") as m_sem,
        nc.semaphore("r_sem") as r_sem,
        nc.semaphore("o_sem") as o_sem,
    ):
        x = xtt[:, 0:F]  # pred logits
        tf = xtt[:, F:FX2]  # target as f32

        # The whole kernel lives in the single `main` basic block: walrus
        # assigns activation-table sets per basic block, so one block means
        # one exp+ln table set, loaded once at the ungated dummy activation
        # below — hidden under the input DMA. It also skips the per-engine
        # block-branch hops. Each engine's sequencer executes only its own
        # instructions, in emission order; semaphores order the dataflow.

        # input DMAs split across BOTH HWDGE rings: x's two quarters-of-the-
        # buffer halves load in parallel (sync ring + scalar ring) so the
        # ACT chain's gating input arrives earlier; t follows on sync
        HF = F // 2
        nc.sync.dma_start(out=xtt[:, 0:HF], in_=xt[:, 0:HF]).then_inc(x_sem, 16)
        nc.scalar.dma_start(out=xtt[:, HF:F], in_=xt[:, HF:F]).then_inc(x2_sem, 16)
        nc.sync.dma_start(out=xtt[:, F:FX2], in_=xt[:, F:FX2]).then_inc(t_sem, 16)

        # scalar engine: dummy activation forces the PWP table load now
        # (inserted before the first ACTIVATE, i.e. after the dma issue);
        # then softplus(x) = ln(1 + exp(x)) with a row-sum accumulator
        nc.scalar.activation(trash[:, :], zeros128, mybir.ActivationFunctionType.Exp)
        nc.scalar.wait_ge(x_sem, 16)
        nc.scalar.wait_ge(x2_sem, 16)
        nc.scalar.activation(e[:, :], x, mybir.ActivationFunctionType.Exp)
        # same-engine RAW on `e`: flush the ACT pipeline before Ln reads it
        # (a bare drain() fails walrus codegen; give it a sem update)
        nc.scalar.drain().then_inc(s_sem, 1)
        nc.scalar.wait_ge(s_sem, 1)
        nc.scalar.activation(
            l[:, :],
            e[:, :],
            mybir.ActivationFunctionType.Ln,
            bias=1.0,
            accum_out=sums[:, 0:1],
        ).then_inc(a_sem, 1)

        # vector engine: xz = (x * 1.0) * tf ; sums[:,1] = row-sum(xz)
        # (tensor_tensor_reduce is broken in this walrus build — "ISA wrong
        # length" — scalar_tensor_tensor+accum is the working equivalent.)
        nc.vector.wait_ge(x_sem, 16)
        nc.vector.wait_ge(x2_sem, 16)
        nc.vector.wait_ge(t_sem, 16)
        nc.vector.scalar_tensor_tensor(
            out=xz[:, :],
            in0=x,
            scalar=1.0,
            in1=tf,
            op0=mybir.AluOpType.mult,
            op1=mybir.AluOpType.mult,
            accum_out=sums[:, 1:2],
        ).then_inc(v_sem, 1)

        # tensor engine: warm-up matmul under the DMA shadow, then collapse
        # the 128 partition partials: [1,2] = ones[128,1].T @ sums[128,2]
        nc.tensor.matmul(ps_warm[:, 0:1], ones128, ones128, start=True, stop=True)
        nc.tensor.wait_ge(a_sem, 1)
        nc.tensor.wait_ge(v_sem, 1)
        nc.tensor.matmul(
            ps[:, :], ones128, sums[:, :], start=True, stop=True
        ).then_inc(m_sem, 1)

        # bounce the matmul result PSUM -> SBUF (DMA can't read PSUM)
        nc.vector.wait_ge(m_sem, 1)
        nc.vector.tensor_copy(res[:, :], ps[:, :]).then_inc(r_sem, 1)

        # output DMA: one 8-byte descriptor; no completion wait and no
        # explicit end barrier — the compiler-injected teardown (per-engine
        # drains + semaphore-file reset + two barrier rounds, ~7 us) retires
        # the in-flight 8-byte write long before the NEFF ends
        nc.sync.wait_ge(r_sem, 1)
        nc.sync.dma_start(out=out[:, :], in_=res[:, :]).then_inc(o_sem, 16)

    return nc


def kernel(pred: np.ndarray, target: np.ndarray) -> np.ndarray:
    from concourse.bass_utils import run_bass_kernel_spmd

    pred = np.asarray(pred, dtype=np.float32)
    target = np.asarray(target)

    xt = np.empty((B, P, FX2), dtype=np.float32)
    xt[:, :, :F] = pred.reshape(B, P, F)
    xt[:, :, F:] = target.reshape(B, P, F).astype(np.float32)

    nc = _build_nc()
    in_maps = [{"xt": xt[b]} for b in range(B)]
    res = run_bass_kernel_spmd(nc, in_maps, list(range(N_CORES)))

    total = 0.0
    for r in res.results:
        o = r["out"].astype(np.float64)
        total += o[0, 0] - o[0, 1]
    return np.array(total / (B * H * W), dtype=np.float32)


# revision 29
# speedup vs baseline: 1.0506x; 1.0506x over previous
"""Trainium2 Bass kernel for nn_BoundaryLoss_49306224558104.

Math note: in the reference, every pixel is either foreground (where
neg = edt(~fg) is exactly 0) or background (where pos = edt(fg) is
exactly 0), so min(pos, neg) == 0 at every pixel and dist_map is
identically zero (bitwise-exact in f32: the EDT of a pixel whose own
d0 is 0 takes the y==j / k==i branch with cost 0, and sqrt(0) == 0).
The loss therefore reduces exactly to mean(softplus(x) - x*z) with
x = pred.squeeze(1), z = (target > 0).

Sharding: pure data-parallel — sample b goes to core b (B == 8 ==
n_cores). Per core, the sample's pred (f32) and target (cast to f32
on host) are packed into one [128, 1024] DRAM buffer, DMA'd on the
sync HWDGE ring (the scalar ring stays free so the ACT PWP-table
load, forced early by a dummy activation, doesn't queue behind a
transfer). softplus(x) = ln(1 + exp(x)) on the scalar engine (inputs
are N(0,1) logits, |x| << 88, so the direct form neither overflows
nor loses precision; this build's act tables have exp+ln in one set
but no softplus table at all). Row sums come from the activation /
scalar_tensor_tensor accumulators; a ones-vector matmul on the
tensor engine collapses the 128 partition partials to a single
[1, 2] PSUM value so the output DMA is one 8-byte descriptor (a
[128, 1] per-partition DMA costs ~7 us in descriptor processing).
The compiler-injected teardown drains retire the in-flight output
DMA (~2 us HBM write receipt). Host combines the 8 x [1, 2] partials
into the scalar mean. Measured: ~15 us HW exec (from ~23.7 us for
the naive version), relative error 0.0 vs the f32 reference value.
"""

import numpy as np

B, H, W = 8, 256, 256
P, F = 128, 512  # H*W == P*F
FX2 = 2 * F
N_CORES = 8


def _build_nc():
    import concourse.bass as bass
    import concourse.mybir as mybir

    nc = bass.Bass(trn_type="TRN2")

    xt = nc.declare_dram_parameter("xt", [P, FX2], mybir.dt.float32, isOutput=False)
    out = nc.declare_dram_parameter("out", [1, 2], mybir.dt.float32, isOutput=True)

    zeros128 = nc.const_aps.aps[(mybir.dt.float32, 0.0)]  # [128,1] framework const
    ones128 = nc.const_aps.aps[(mybir.dt.float32, 1.0)]  # [128,1] framework const

    with (
        nc.sbuf_tensor("xtt", [P, FX2], mybir.dt.float32) as xtt,
        nc.sbuf_tensor("e", [P, F], mybir.dt.float32) as e,
        nc.sbuf_tensor("l", [P, F], mybir.dt.float32) as l,
        nc.sbuf_tensor("xz", [P, F], mybir.dt.float32) as xz,
        nc.sbuf_tensor("sums", [P, 2], mybir.dt.float32) as sums,
        nc.sbuf_tensor("trash", [P, 1], mybir.dt.float32) as trash,
        nc.sbuf_tensor("res", [1, 2], mybir.dt.float32) as res,
        nc.psum_tensor("ps", [1, 2], mybir.dt.float32) as ps,
        nc.psum_tensor("ps_warm", [1, 2], mybir.dt.float32) as ps_warm,
        nc.semaphore("x_sem") as x_sem,
        nc.semaphore("t_sem") as t_sem,
        nc.semaphore("s_sem") as s_sem,
        nc.semaphore("a_sem") as a_sem,
        nc.semaphore("v_sem") as v_sem,
        nc.semaphore("m_sem") as m_sem,
        nc.semaphore("r_sem") as r_sem,
        nc.semaphore("o_sem") as o_sem,
    ):
        x = xtt[:, 0:F]  # pred logits
        tf = xtt[:, F:FX2]  # target as f32

        # The whole kernel lives in the single `main` basic block: walrus
        # assigns activation-table sets per basic block, so one block means
        # one exp+ln table set, loaded once at the ungated dummy activation
        # below — hidden under the input DMA. It also skips the per-engine
        # block-branch hops. Each engine's sequencer executes only its own
        # instructions, in emission order; semaphores order the dataflow.

        # input DMAs on the sync HWDGE ring (scalar's ring is left free so
        # the ACT table load doesn't queue behind a transfer)
        nc.sync.dma_start(out=xtt[:, 0:F], in_=xt[:, 0:F]).then_inc(x_sem, 16)
        nc.sync.dma_start(out=xtt[:, F:FX2], in_=xt[:, F:FX2]).then_inc(t_sem, 16)

        # scalar engine: dummy activation forces the PWP table load now;
        # then softplus(x) = ln(1 + exp(x)) with a row-sum accumulator
        nc.scalar.activation(trash[:, :], zeros128, mybir.ActivationFunctionType.Exp)
        nc.scalar.wait_ge(x_sem, 16)
        nc.scalar.activation(e[:, :], x, mybir.ActivationFunctionType.Exp)
        # same-engine RAW on `e`: flush the ACT pipeline before Ln reads it
        # (a bare drain() fails walrus codegen; give it a sem update)
        nc.scalar.drain().then_inc(s_sem, 1)
        nc.scalar.wait_ge(s_sem, 1)
        nc.scalar.activation(
            l[:, :],
            e[:, :],
            mybir.ActivationFunctionType.Ln,
            bias=1.0,
            accum_out=sums[:, 0:1],
        ).then_inc(a_sem, 1)

        # vector engine: xz = (x * 1.0) * tf ; sums[:,1] = row-sum(xz)
        # (tensor_tensor_reduce is broken in this walrus build — "ISA wrong
        # length" — scalar_tensor_tensor+accum is the working equivalent.)
        nc.vector.wait_ge(x_sem, 16)
        nc.vector.wait_ge(t_sem, 16)
        nc.vector.scalar_tensor_tensor(
            out=xz[:, :],
            in0=x,
            scalar=1.0,
            in1=tf,
            op0=mybir.AluOpType.mult,
            op1=mybir.AluOpType.mult,
            accum_out=sums[:, 1:2],
        ).then_inc(v_sem, 1)

        # tensor engine: warm-up matmul under the DMA shadow, then collapse
        # the 128 partition partials: [1,2] = ones[128,1].T @ sums[128,2]
        nc.tensor.matmul(ps_warm[:, 0:1], ones128, ones128, start=True, stop=True)
        nc.tensor.wait_ge(a_sem, 1)
        nc.tensor.wait_ge(v_sem, 1)
        nc.tensor.matmul(
            ps[:, :], ones128, sums[:, :], start=True, stop=True
        ).then_inc(m_sem, 1)

        # bounce the matmul result PSUM -> SBUF (DMA can't read PSUM)
        nc.vector.wait_ge(m_sem, 1)
        nc.vector.tensor_copy(res[:, :], ps[:, :]).then_inc(r_sem, 1)

        # output DMA: one 8-byte descriptor; no completion wait and no
        # explicit end barrier — the compiler-injected teardown (per-engine
        # drains + semaphore-file reset + two barrier rounds, ~7 us) retires
        # the in-flight 8-byte write long before the NEFF ends
        nc.sync.wait_ge(r_sem, 1)
        nc.sync.dma_start(out=out[:, :], in_=res[:, :]).then_inc(o_sem, 16)

    return nc


def kernel(pred: np.ndarray, target: np.ndarray) -> np.ndarray:
    from concourse.bass_utils import run_bass_kernel_spmd

    pred = np.asarray(pred, dtype=np.float32)
    target = np.asarray(target)

    xt = np.empty((B, P, FX2), dtype=np.float32)
    xt[:, :, :F] = pred.reshape(B, P, F)
    xt[:, :, F:] = target.reshape(B, P, F).astype(np.float32)

    nc = _build_nc()
    in_maps = [{"xt": xt[b]} for b in range(B)]
    res = run_bass_kernel_spmd(nc, in_maps, list(range(N_CORES)))

    total = 0.0
    for r in res.results:
        o = r["out"].astype(np.float64)
        total += o[0, 0] - o[0, 1]
    return np.array(total / (B * H * W), dtype=np.float32)


# revision 31
# speedup vs baseline: 1.0773x; 1.0254x over previous
"""Trainium2 Bass kernel for nn_BoundaryLoss_49306224558104.

Math note: in the reference, every pixel is either foreground (where
neg = edt(~fg) is exactly 0) or background (where pos = edt(fg) is
exactly 0), so min(pos, neg) == 0 at every pixel and dist_map is
identically zero (bitwise-exact in f32: the EDT of a pixel whose own
d0 is 0 takes the y==j / k==i branch with cost 0, and sqrt(0) == 0).
The loss therefore reduces exactly to mean(softplus(x) - x*z) with
x = pred.squeeze(1), z = (target > 0).

Sharding: pure data-parallel — sample b goes to core b (B == 8 ==
n_cores). Per core, the sample's pred (f32) and target (cast to f32
on host) are packed into one [128, 1024] DRAM buffer, DMA'd on the
sync HWDGE ring (the scalar ring stays free so the ACT PWP-table
load, forced early by a dummy activation, doesn't queue behind a
transfer). softplus(x) = ln(1 + exp(x)) on the scalar engine (inputs
are N(0,1) logits, |x| << 88, so the direct form neither overflows
nor loses precision; this build's act tables have exp+ln in one set
but no softplus table at all). Row sums come from the activation /
scalar_tensor_tensor accumulators; a ones-vector matmul on the
tensor engine collapses the 128 partition partials to a single
[1, 2] PSUM value so the output DMA is one 8-byte descriptor (a
[128, 1] per-partition DMA costs ~7 us in descriptor processing).
The compiler-injected teardown drains retire the in-flight output
DMA (~2 us HBM write receipt). Host combines the 8 x [1, 2] partials
into the scalar mean. Measured: ~15 us HW exec (from ~23.7 us for
the naive version), relative error 0.0 vs the f32 reference value.
"""

import numpy as np

B, H, W = 8, 256, 256
P, F = 128, 512  # H*W == P*F
FX2 = 2 * F
N_CORES = 8


def _build_nc():
    import concourse.bass as bass
    import concourse.mybir as mybir

    nc = bass.Bass(trn_type="TRN2")

    xt = nc.declare_dram_parameter("xt", [P, FX2], mybir.dt.float32, isOutput=False)
    out = nc.declare_dram_parameter("out", [1, 2], mybir.dt.float32, isOutput=True)

    zeros128 = nc.const_aps.aps[(mybir.dt.float32, 0.0)]  # [128,1] framework const
    ones128 = nc.const_aps.aps[(mybir.dt.float32, 1.0)]  # [128,1] framework const

    with (
        nc.sbuf_tensor("xtt", [P, FX2], mybir.dt.float32) as xtt,
        nc.sbuf_tensor("e", [P, F], mybir.dt.float32) as e,
        nc.sbuf_tensor("l", [P, F], mybir.dt.float32) as l,
        nc.sbuf_tensor("xz", [P, F], mybir.dt.float32) as xz,
        nc.sbuf_tensor("sums", [P, 2], mybir.dt.float32) as sums,
        nc.sbuf_tensor("trash", [P, 1], mybir.dt.float32) as trash,
        nc.sbuf_tensor("res", [1, 2], mybir.dt.float32) as res,
        nc.psum_tensor("ps", [1, 2], mybir.dt.float32) as ps,
        nc.psum_tensor("ps_warm", [1, 2], mybir.dt.float32) as ps_warm,
        nc.semaphore("x_sem") as x_sem,
        nc.semaphore("t_sem") as t_sem,
        nc.semaphore("s_sem") as s_sem,
        nc.semaphore("a_sem") as a_sem,
        nc.semaphore("v_sem") as v_sem,
        nc.semaphore("m_sem") as m_sem,
        nc.semaphore("r_sem") as r_sem,
        nc.semaphore("o_sem") as o_sem,
    ):
        x = xtt[:, 0:F]  # pred logits
        tf = xtt[:, F:FX2]  # target as f32

        # The whole kernel lives in the single `main` basic block: walrus
        # assigns activation-table sets per basic block, so one block means
        # one exp+ln table set, loaded once at the ungated dummy activation
        # below — hidden under the input DMA. It also skips the per-engine
        # block-branch hops. Each engine's sequencer executes only its own
        # instructions, in emission order; semaphores order the dataflow.

        # input DMAs on the sync HWDGE ring (scalar's ring is left free so
        # the ACT table load doesn't queue behind a transfer)
        nc.sync.dma_start(out=xtt[:, 0:F], in_=xt[:, 0:F]).then_inc(x_sem, 16)
        nc.sync.dma_start(out=xtt[:, F:FX2], in_=xt[:, F:FX2]).then_inc(t_sem, 16)

        # scalar engine: dummy activation forces the PWP table load now;
        # then softplus(x) = ln(1 + exp(x)) with a row-sum accumulator
        nc.scalar.activation(trash[:, :], zeros128, mybir.ActivationFunctionType.Exp)
        nc.scalar.wait_ge(x_sem, 16)
        nc.scalar.activation(e[:, :], x, mybir.ActivationFunctionType.Exp)
        # same-engine RAW on `e`: flush the ACT pipeline before Ln reads it
        # (a bare drain() fails walrus codegen; give it a sem update)
        nc.scalar.drain().then_inc(s_sem, 1)
        nc.scalar.wait_ge(s_sem, 1)
        nc.scalar.activation(
            l[:, :],
            e[:, :],
            mybir.ActivationFunctionType.Ln,
            bias=1.0,
            accum_out=sums[:, 0:1],
        ).then_inc(a_sem, 1)

        # vector engine: xz = (x * 1.0) * tf ; sums[:,1] = row-sum(xz)
        # (tensor_tensor_reduce is broken in this walrus build — "ISA wrong
        # length" — scalar_tensor_tensor+accum is the working equivalent.)
        nc.vector.wait_ge(x_sem, 16)
        nc.vector.wait_ge(t_sem, 16)
        nc.vector.scalar_tensor_tensor(
            out=xz[:, :],
            in0=x,
            scalar=1.0,
            in1=tf,
            op0=mybir.AluOpType.mult,
            op1=mybir.AluOpType.mult,
            accum_out=sums[:, 1:2],
        ).then_inc(v_sem, 1)

        # tensor engine: warm-up matmul under the DMA shadow, then collapse
        # the 128 partition partials: [1,2] = ones[128,1].T @ sums[128,2]
        nc.tensor.matmul(ps_warm[:, 0:1], ones128, ones128, start=True, stop=True)
        nc.tensor.wait_ge(a_sem, 1)
        nc.tensor.wait_ge(v_sem, 1)
        nc.tensor.matmul(
            ps[:, :], ones128, sums[:, :], start=True, stop=True
        ).then_inc(m_sem, 1)

        # bounce the matmul result PSUM -> SBUF (DMA can't read PSUM)
        nc.vector.wait_ge(m_sem, 1)
        nc.vector.tensor_copy(res[:, :], ps[:, :]).then_inc(r_sem, 1)

        # output DMA: one 8-byte descriptor with its (mandatory) completion
        # semaphore, but no completion wait and no explicit end barrier —
        # the compiler-injected teardown (per-engine drains + semaphore-file
        # reset + two barrier rounds, ~7 us) retires the in-flight 8-byte
        # write long before the NEFF ends
        nc.sync.wait_ge(r_sem, 1)
        nc.sync.dma_start(out=out[:, :], in_=res[:, :]).then_inc(o_sem, 16)

    return nc


def kernel(pred: np.ndarray, target: np.ndarray) -> np.ndarray:
    from concourse.bass_utils import run_bass_kernel_spmd

    pred = np.asarray(pred, dtype=np.float32)
    target = np.asarray(target)

    xt = np.empty((B, P, FX2), dtype=np.float32)
    xt[:, :, :F] = pred.reshape(B, P, F)
    xt[:, :, F:] = target.reshape(B, P, F).astype(np.float32)

    nc = _build_nc()
    in_maps = [{"xt": xt[b]} for b in range(B)]
    res = run_bass_kernel_spmd(nc, in_maps, list(range(N_CORES)))

    total = 0.0
    for r in res.results:
        o = r["out"].astype(np.float64)
        total += o[0, 0] - o[0, 1]
    return np.array(total / (B * H * W), dtype=np.float32)


# revision 33
# speedup vs baseline: 1.0835x; 1.0058x over previous
"""Trainium2 Bass kernel for nn_BoundaryLoss_49306224558104.

Math note: in the reference, every pixel is either foreground (where
neg = edt(~fg) is exactly 0) or background (where pos = edt(fg) is
exactly 0), so min(pos, neg) == 0 at every pixel and dist_map is
identically zero (bitwise-exact in f32: the EDT of a pixel whose own
d0 is 0 takes the y==j / k==i branch with cost 0, and sqrt(0) == 0).
The loss therefore reduces exactly to mean(softplus(x) - x*z) with
x = pred.squeeze(1), z = (target > 0).

Sharding: pure data-parallel — sample b goes to core b (B == 8 ==
n_cores). Per core, the sample's pred (f32) and target (cast to f32
on host) are packed into one [128, 1024] DRAM buffer, DMA'd on the
sync HWDGE ring (the scalar ring stays free so the ACT PWP-table
load, forced early by a dummy activation, doesn't queue behind a
transfer). softplus(x) = ln(1 + exp(x)) on the scalar engine (inputs
are N(0,1) logits, |x| << 88, so the direct form neither overflows
nor loses precision; this build's act tables have exp+ln in one set
but no softplus table at all). Row sums come from the activation /
scalar_tensor_tensor accumulators; a ones-vector matmul on the
tensor engine collapses the 128 partition partials to a single
[1, 2] PSUM value so the output DMA is one 8-byte descriptor (a
[128, 1] per-partition DMA costs ~7 us in descriptor processing).
The compiler-injected teardown drains retire the in-flight output
DMA (~2 us HBM write receipt). Host combines the 8 x [1, 2] partials
into the scalar mean. Measured: ~15 us HW exec (from ~23.7 us for
the naive version), relative error 0.0 vs the f32 reference value.
"""

import numpy as np

B, H, W = 8, 256, 256
P, F = 128, 512  # H*W == P*F
FX2 = 2 * F
N_CORES = 8


def _build_nc():
    import concourse.bass as bass
    import concourse.mybir as mybir

    nc = bass.Bass(trn_type="TRN2")

    xt = nc.declare_dram_parameter("xt", [P, FX2], mybir.dt.float32, isOutput=False)
    out = nc.declare_dram_parameter("out", [1, 2], mybir.dt.float32, isOutput=True)

    zeros128 = nc.const_aps.aps[(mybir.dt.float32, 0.0)]  # [128,1] framework const
    ones128 = nc.const_aps.aps[(mybir.dt.float32, 1.0)]  # [128,1] framework const

    with (
        nc.sbuf_tensor("xtt", [P, FX2], mybir.dt.float32) as xtt,
        nc.sbuf_tensor("e", [P, F], mybir.dt.float32) as e,
        nc.sbuf_tensor("l", [P, F], mybir.dt.float32) as l,
        nc.sbuf_tensor("xz", [P, F], mybir.dt.float32) as xz,
        nc.sbuf_tensor("sums", [P, 2], mybir.dt.float32) as sums,
        nc.sbuf_tensor("trash", [P, 1], mybir.dt.float32) as trash,
        nc.sbuf_tensor("res", [1, 2], mybir.dt.float32) as res,
        nc.psum_tensor("ps", [1, 2], mybir.dt.float32) as ps,
        nc.psum_tensor("ps_warm", [1, 2], mybir.dt.float32) as ps_warm,
        nc.semaphore("x_sem") as x_sem,
        nc.semaphore("t_sem") as t_sem,
        nc.semaphore("s_sem") as s_sem,
        nc.semaphore("a_sem") as a_sem,
        nc.semaphore("v_sem") as v_sem,
        nc.semaphore("m_sem") as m_sem,
        nc.semaphore("r_sem") as r_sem,
        nc.semaphore("o_sem") as o_sem,
    ):
        x = xtt[:, 0:F]  # pred logits
        tf = xtt[:, F:FX2]  # target as f32

        # The whole kernel lives in the single `main` basic block: walrus
        # assigns activation-table sets per basic block, so one block means
        # one exp+ln table set, loaded once at the ungated dummy activation
        # below — hidden under the input DMA. It also skips the per-engine
        # block-branch hops. Each engine's sequencer executes only its own
        # instructions, in emission order; semaphores order the dataflow.

        # input DMAs on the sync HWDGE ring (scalar's ring is left free so
        # the ACT table load doesn't queue behind a transfer)
        nc.sync.dma_start(out=xtt[:, 0:F], in_=xt[:, 0:F]).then_inc(x_sem, 16)
        nc.sync.dma_start(out=xtt[:, F:FX2], in_=xt[:, F:FX2]).then_inc(t_sem, 16)

        # scalar engine: dummy activation forces the PWP table load now;
        # then softplus(x) = ln(1 + exp(x)) with a row-sum accumulator
        nc.scalar.activation(trash[:, :], zeros128, mybir.ActivationFunctionType.Exp)
        nc.scalar.wait_ge(x_sem, 16)
        nc.scalar.activation(e[:, :], x, mybir.ActivationFunctionType.Exp)
        # same-engine RAW on `e`: flush the ACT pipeline before Ln reads it
        # (a bare drain() fails walrus codegen; give it a sem update)
        nc.scalar.drain().then_inc(s_sem, 1)
        nc.scalar.wait_ge(s_sem, 1)
        nc.scalar.activation(
            l[:, :],
            e[:, :],
            mybir.ActivationFunctionType.Ln,
            bias=1.0,
            accum_out=sums[:, 0:1],
        ).then_inc(a_sem, 1)

        # vector engine: xz = (x * 1.0) * tf ; sums[:,1] = row-sum(xz)
        # (tensor_tensor_reduce is broken in this walrus build — "ISA wrong
        # length" — scalar_tensor_tensor+accum is the working equivalent.)
        nc.vector.wait_ge(x_sem, 16)
        nc.vector.wait_ge(t_sem, 16)
        nc.vector.scalar_tensor_tensor(
            out=xz[:, :],
            in0=x,
            scalar=1.0,
            in1=tf,
            op0=mybir.AluOpType.mult,
            op1=mybir.AluOpType.mult,
            accum_out=sums[:, 1:2],
        ).then_inc(v_sem, 1)

        # tensor engine: warm-up matmul under the DMA shadow, then collapse
        # the 128 partition partials column-by-column — the xz column is
        # ready (v_sem) before the softplus accumulator (a_sem), so its
        # matmul isn't gated on the ACT chain
        nc.tensor.matmul(ps_warm[:, 0:1], ones128, ones128, start=True, stop=True)
        nc.tensor.wait_ge(v_sem, 1)
        nc.tensor.matmul(
            ps[:, 1:2], ones128, sums[:, 1:2], start=True, stop=True
        ).then_inc(m_sem, 1)
        nc.tensor.wait_ge(a_sem, 1)
        nc.tensor.matmul(
            ps[:, 0:1], ones128, sums[:, 0:1], start=True, stop=True
        ).then_inc(m_sem, 1)

        # bounce the matmul result PSUM -> SBUF (DMA can't read PSUM)
        nc.vector.wait_ge(m_sem, 2)
        nc.vector.tensor_copy(res[:, :], ps[:, :]).then_inc(r_sem, 1)

        # output DMA: one 8-byte descriptor with its (mandatory) completion
        # semaphore, but no completion wait and no explicit end barrier —
        # the compiler-injected teardown (per-engine drains + semaphore-file
        # reset + two barrier rounds, ~7 us) retires the in-flight 8-byte
        # write long before the NEFF ends
        nc.sync.wait_ge(r_sem, 1)
        nc.sync.dma_start(out=out[:, :], in_=res[:, :], single_packet=True).then_inc(
            o_sem, 16
        )

    return nc


def kernel(pred: np.ndarray, target: np.ndarray) -> np.ndarray:
    from concourse.bass_utils import run_bass_kernel_spmd

    pred = np.asarray(pred, dtype=np.float32)
    target = np.asarray(target)

    xt = np.empty((B, P, FX2), dtype=np.float32)
    xt[:, :, :F] = pred.reshape(B, P, F)
    xt[:, :, F:] = target.reshape(B, P, F).astype(np.float32)

    nc = _build_nc()
    in_maps = [{"xt": xt[b]} for b in range(B)]
    res = run_bass_kernel_spmd(nc, in_maps, list(range(N_CORES)))

    total = 0.0
    for r in res.results:
        o = r["out"].astype(np.float64)
        total += o[0, 0] - o[0, 1]
    return np.array(total / (B * H * W), dtype=np.float32)
